# revision 14
# baseline (speedup 1.0000x reference)
"""MoE grouped-linear (ragged matmul + bias) on 8 TRN2 NeuronCores.

Expert-parallel sharding: core e computes tokens of expert e:
    out_e = X_e[cap, 2048] @ W_e[2048, 8192] + bias
Tokens are pre-sorted by expert (contiguous groups), so the "all-to-all"
is a free host-side slice/concat. No on-device collectives.

Per-core kernel: X_e^T fully resident in SBUF, W_e streamed in 512-col
chunks, PSUM accumulation over K=16 k-tiles, bias fused into the PSUM
eviction on the vector engine. Matmuls run in float32r (TF32-like,
1 cycle/row for moving dim >= 256 vs 4 cycles/row for plain fp32).
"""

import numpy as np

E, IN, OUT = 8, 2048, 8192
P = 128
NTILE = 512

_cache = {}


def _build(cap, dtype_name="float32r", reps=1, mode="full", ntile=None):
    import contextlib

    import concourse.mybir as mybir
    import concourse.tile as tile
    from concourse import bacc

    mm_dt = getattr(mybir.dt, dtype_name)
    nt = ntile or NTILE
    KT = IN // P            # 16 k-tiles
    MT = cap // P           # m-tiles per core
    NT = OUT // nt          # n-tiles

    nc = bacc.Bacc(None, target_bir_lowering=False, debug=False)
    with tile.TileContext(nc) as tc:
        with tc.tile_pool(name="dram", bufs=1, space="DRAM") as dram:
            # xt_d[mi, p, k, j] = X[mi*P + j, k*P + p] — per-mi contiguous
            # 1MB slices so the first matmul group can start after ~1MB of DMA
            xt_d = dram.tile((MT, P, KT, P), mm_dt, kind="ExternalInput")
            w_d = dram.tile((P, KT, OUT), mm_dt, kind="ExternalInput")
            bias_d = dram.tile((P, OUT), mybir.dt.float32, kind="ExternalInput")
            out_d = dram.tile((P, MT, OUT), mybir.dt.float32, kind="ExternalOutput")

            with tc.tile_pool(name="resident", bufs=1) as res_pool, \
                 tc.tile_pool(name="wchunk", bufs=2) as w_pool, \
                 tc.tile_pool(name="evict", bufs=6) as o_pool, \
                 tc.tile_pool(name="acc", bufs=6, space="PSUM") as ps_pool:
                loop = tc.For_i(0, reps, 1) if reps > 1 else contextlib.nullcontext()
                with loop:
                    # W stream owns the sync queue; X^T + bias load in
                    # parallel on the gpsimd queue, first-needed first.
                    w_sbs = [None] * NT
                    w_sbs[0] = w_pool.tile([P, KT, nt], mm_dt, tag="w",
                                           name="w_sb0")
                    nc.sync.dma_start(w_sbs[0][:], w_d[:, :, 0:nt])

                    xt_sb = [res_pool.tile([P, KT, P], mm_dt, tag=f"xt{mi}",
                                           name=f"xt_sb{mi}")
                             for mi in range(MT)]
                    bias_sb = res_pool.tile([P, OUT], mybir.dt.float32)
                    nc.gpsimd.dma_start(xt_sb[0][:], xt_d[0])
                    nc.gpsimd.dma_start(bias_sb[:], bias_d[:])
                    for mi in range(1, MT):
                        nc.gpsimd.dma_start(xt_sb[mi][:], xt_d[mi])

                    for ni in range(NT):
                        w_sb = w_sbs[ni]
                        if w_sb is None and mode in ("mm_only", "same_w"):
                            w_sb = w_sbs[0]
                        elif w_sb is None:
                            w_sb = w_pool.tile([P, KT, nt], mm_dt, tag="w",
                                               name=f"w_sb{ni}")
                            nc.sync.dma_start(
                                w_sb[:], w_d[:, :, ni * nt:(ni + 1) * nt])
                        for mi in range(MT):
                            ps = ps_pool.tile([P, nt], mybir.dt.float32)
                            for k in range(KT):
                                nc.tensor.matmul(
                                    ps[:],
                                    lhsT=xt_sb[0][:, 0, :] if mode == "same_w"
                                    else xt_sb[mi][:, k, :],
                                    rhs=w_sb[:, k, :],
                                    start=(k == 0),
                                    stop=(k == KT - 1),
                                )
                            if mode in ("mm_only", "same_w") and not (ni == NT - 1 and mi == MT - 1):
                                continue
                            o_sb = o_pool.tile([P, nt], mybir.dt.float32)
                            nc.vector.tensor_add(
                                out=o_sb[:], in0=ps[:],
                                in1=bias_sb[:, ni * nt:(ni + 1) * nt])
                            nc.sync.dma_start(
                                out_d[:, mi, ni * nt:(ni + 1) * nt], o_sb[:])
    nc.compile()
    names = dict(xt=xt_d.name, w=w_d.name, bias=bias_d.name, out=out_d.name)
    return nc, names


def _get(cap, dtype_name="float32r", reps=1, mode="full", ntile=None):
    key = (cap, dtype_name, reps, mode, ntile)
    if key not in _cache:
        _cache[key] = _build(cap, dtype_name, reps, mode, ntile)
    return _cache[key]


def kernel(inputs, weight, group_sizes, bias):
    from concourse.bass_utils import run_bass_kernel_spmd

    M = inputs.shape[0]
    gs = np.asarray(group_sizes, dtype=np.int64)
    # per-token expert id exactly as the reference's jnp.repeat(...,
    # total_repeat_length=M): truncate or pad with the last expert id
    ids = np.repeat(np.arange(E), gs)
    ids = ids[:M] if len(ids) >= M else np.concatenate(
        [ids, np.full(M - len(ids), E - 1)])
    counts = np.bincount(ids, minlength=E)
    starts = np.concatenate([[0], np.cumsum(counts)])[:E]

    cap = max(P, int(-(-counts.max() // P) * P))
    nc, names = _get(cap)

    x = np.ascontiguousarray(inputs, dtype=np.float32)
    w = np.ascontiguousarray(weight, dtype=np.float32)
    bias_rep = np.ascontiguousarray(
        np.broadcast_to(np.asarray(bias, np.float32), (P, OUT)))

    in_maps = []
    for e in range(E):
        xe = x[starts[e]:starts[e] + counts[e]]
        if xe.shape[0] < cap:
            xe = np.concatenate(
                [xe, np.zeros((cap - xe.shape[0], IN), np.float32)])
        # [cap, IN] -> (MT, P, KT, P): xt[mi, p, k, j] = X[mi*P+j, k*P+p]
        xt = np.ascontiguousarray(
            xe.reshape(cap // P, P, IN // P, P).transpose(0, 3, 2, 1))
        # [IN, OUT] -> (P, KT, OUT): wt[p, a, n] = W[a*P+p, n]
        we = np.ascontiguousarray(
            w[e].reshape(IN // P, P, OUT).transpose(1, 0, 2))
        in_maps.append({names["xt"]: xt, names["w"]: we,
                        names["bias"]: bias_rep})

    res = run_bass_kernel_spmd(nc, in_maps, core_ids=list(range(E)))
    out = np.empty((M, OUT), dtype=np.float32)
    for e in range(E):
        oe = res.results[e][names["out"]]          # (P, cap//P, OUT)
        oe = oe.transpose(1, 0, 2).reshape(cap, OUT)
        out[starts[e]:starts[e] + counts[e]] = oe[:counts[e]]
    return out


# revision 15
# speedup vs baseline: 1.2784x; 1.2784x over previous
"""MoE grouped-linear (ragged matmul + bias) on 8 TRN2 NeuronCores.

Expert-parallel sharding: core e computes tokens of expert e:
    out_e = X_e[cap, 2048] @ W_e[2048, 8192] + bias
Tokens are pre-sorted by expert (contiguous groups), so the "all-to-all"
is a free host-side slice/concat. No on-device collectives.

Per-core kernel: X_e^T fully resident in SBUF (loaded in per-mi 1MB
slices on the gpsimd DMA queue so the first matmul group starts ~13us
in), W_e streamed in 512-col chunks on the sync queue, PSUM
accumulation over K=16 k-tiles, bias fused into the PSUM eviction on
the vector engine. Matmuls run in float32r (TF32-like multiply, fp32
accumulate: 1 cycle/row for moving dim >= 256 vs 4 cycles/row for
plain fp32; rel err ~1.6e-4 on this problem).

Measured on TRN2 via reps-slope (NTFF profiling unavailable under
axon): ~575-580 us/core, vs a ~573 us streaming floor at the observed
~2.0 GHz effective PE clock (512-row matmul ~258 ns + ~22 ns issue
overhead; weight-load count does not matter — verified by a
same-stationary variant). bf16 measures the same, fp8 would halve it
but fails accuracy.
"""

import numpy as np

E, IN, OUT = 8, 2048, 8192
P = 128
NTILE = 512

_cache = {}


def _build(cap, dtype_name="float32r", reps=1, mode="full", ntile=None):
    import contextlib

    import concourse.mybir as mybir
    import concourse.tile as tile
    from concourse import bacc

    mm_dt = getattr(mybir.dt, dtype_name)
    nt = ntile or NTILE
    KT = IN // P            # 16 k-tiles
    MT = cap // P           # m-tiles per core
    NT = OUT // nt          # n-tiles

    nc = bacc.Bacc(None, target_bir_lowering=False, debug=False)
    with tile.TileContext(nc) as tc:
        with tc.tile_pool(name="dram", bufs=1, space="DRAM") as dram:
            # xt_d[mi, p, k, j] = X[mi*P + j, k*P + p] — per-mi contiguous
            # 1MB slices so the first matmul group can start after ~1MB of DMA
            xt_d = dram.tile((MT, P, KT, P), mm_dt, kind="ExternalInput")
            w_d = dram.tile((P, KT, OUT), mm_dt, kind="ExternalInput")
            bias_d = dram.tile((P, OUT), mybir.dt.float32, kind="ExternalInput")
            out_d = dram.tile((P, MT, OUT), mybir.dt.float32, kind="ExternalOutput")

            with tc.tile_pool(name="resident", bufs=1) as res_pool, \
                 tc.tile_pool(name="wchunk", bufs=2) as w_pool, \
                 tc.tile_pool(name="evict", bufs=6) as o_pool, \
                 tc.tile_pool(name="acc", bufs=6, space="PSUM") as ps_pool:
                loop = tc.For_i(0, reps, 1) if reps > 1 else contextlib.nullcontext()
                with loop:
                    # W stream owns the sync queue; X^T + bias load in
                    # parallel on the gpsimd queue, first-needed first.
                    w_sbs = [None] * NT
                    w_sbs[0] = w_pool.tile([P, KT, nt], mm_dt, tag="w",
                                           name="w_sb0")
                    nc.sync.dma_start(w_sbs[0][:], w_d[:, :, 0:nt])

                    xt_sb = [res_pool.tile([P, KT, P], mm_dt, tag=f"xt{mi}",
                                           name=f"xt_sb{mi}")
                             for mi in range(MT)]
                    bias_sb = res_pool.tile([P, OUT], mybir.dt.float32)
                    nc.gpsimd.dma_start(xt_sb[0][:], xt_d[0])
                    nc.gpsimd.dma_start(bias_sb[:], bias_d[:])
                    for mi in range(1, MT):
                        nc.gpsimd.dma_start(xt_sb[mi][:], xt_d[mi])

                    for ni in range(NT):
                        w_sb = w_sbs[ni]
                        if w_sb is None and mode in ("mm_only", "same_w"):
                            w_sb = w_sbs[0]
                        elif w_sb is None:
                            w_sb = w_pool.tile([P, KT, nt], mm_dt, tag="w",
                                               name=f"w_sb{ni}")
                            nc.sync.dma_start(
                                w_sb[:], w_d[:, :, ni * nt:(ni + 1) * nt])
                        for mi in range(MT):
                            ps = ps_pool.tile([P, nt], mybir.dt.float32)
                            for k in range(KT):
                                nc.tensor.matmul(
                                    ps[:],
                                    lhsT=xt_sb[0][:, 0, :] if mode == "same_w"
                                    else xt_sb[mi][:, k, :],
                                    rhs=w_sb[:, k, :],
                                    start=(k == 0),
                                    stop=(k == KT - 1),
                                )
                            if mode in ("mm_only", "same_w") and not (ni == NT - 1 and mi == MT - 1):
                                continue
                            o_sb = o_pool.tile([P, nt], mybir.dt.float32)
                            nc.vector.tensor_add(
                                out=o_sb[:], in0=ps[:],
                                in1=bias_sb[:, ni * nt:(ni + 1) * nt])
                            nc.sync.dma_start(
                                out_d[:, mi, ni * nt:(ni + 1) * nt], o_sb[:])
    nc.compile()
    names = dict(xt=xt_d.name, w=w_d.name, bias=bias_d.name, out=out_d.name)
    return nc, names


def _get(cap, dtype_name="float32r", reps=1, mode="full", ntile=None):
    key = (cap, dtype_name, reps, mode, ntile)
    if key not in _cache:
        _cache[key] = _build(cap, dtype_name, reps, mode, ntile)
    return _cache[key]


def kernel(inputs, weight, group_sizes, bias):
    from concourse.bass_utils import run_bass_kernel_spmd

    M = inputs.shape[0]
    gs = np.asarray(group_sizes, dtype=np.int64)
    # per-token expert id exactly as the reference's jnp.repeat(...,
    # total_repeat_length=M): truncate or pad with the last expert id
    ids = np.repeat(np.arange(E), gs)
    ids = ids[:M] if len(ids) >= M else np.concatenate(
        [ids, np.full(M - len(ids), E - 1)])
    counts = np.bincount(ids, minlength=E)
    starts = np.concatenate([[0], np.cumsum(counts)])[:E]

    cap = max(P, int(-(-counts.max() // P) * P))
    nc, names = _get(cap)

    x = np.ascontiguousarray(inputs, dtype=np.float32)
    w = np.ascontiguousarray(weight, dtype=np.float32)
    bias_rep = np.ascontiguousarray(
        np.broadcast_to(np.asarray(bias, np.float32), (P, OUT)))

    in_maps = []
    for e in range(E):
        xe = x[starts[e]:starts[e] + counts[e]]
        if xe.shape[0] < cap:
            xe = np.concatenate(
                [xe, np.zeros((cap - xe.shape[0], IN), np.float32)])
        # [cap, IN] -> (MT, P, KT, P): xt[mi, p, k, j] = X[mi*P+j, k*P+p]
        xt = np.ascontiguousarray(
            xe.reshape(cap // P, P, IN // P, P).transpose(0, 3, 2, 1))
        # [IN, OUT] -> (P, KT, OUT): wt[p, a, n] = W[a*P+p, n]
        we = np.ascontiguousarray(
            w[e].reshape(IN // P, P, OUT).transpose(1, 0, 2))
        in_maps.append({names["xt"]: xt, names["w"]: we,
                        names["bias"]: bias_rep})

    res = run_bass_kernel_spmd(nc, in_maps, core_ids=list(range(E)))
    out = np.empty((M, OUT), dtype=np.float32)
    for e in range(E):
        oe = res.results[e][names["out"]]          # (P, cap//P, OUT)
        oe = oe.transpose(1, 0, 2).reshape(cap, OUT)
        out[starts[e]:starts[e] + counts[e]] = oe[:counts[e]]
    return out
